# revision 14
# baseline (speedup 1.0000x reference)
"""Trainium2 Bass kernel for DiagonalMicroAttention (nn_DiagonalMicroAttention).

Sharding: data-parallel over batch -- one batch element per NeuronCore
(batch=8, n_cores=8).  All weights replicated; no collectives.

Per-core program (fp32 end-to-end, channels-on-partitions layout):
  x_c [256 chan, 1024 pix]
  1. qT/kT  = W_{q,k} @ x         [512, 1024]  (heads = contiguous 32-row blocks;
                                   attention scale folded into Wq on host)
     vtrans = x^T @ Wv^T          [1024, 256]  (pixel-major, j on partitions)
  2. attention, per 4-head group, per 128-j-tile, per 512-i-half:
       scores^T[j, i] via K=32 matmuls, 4 heads row-tiled into the PE array
       E = exp(scores) one ACT op per [128, 2048] (4 psum banks -> SBUF)
       av^T[dd, i] += vtrans_chunk^T @ E  (4 heads col-tiled, K=128)
       r[h, i]     += ones^T @ E          (softmax denominators via PE)
  3. diag branch: dv[dd] = 0.3/8 * sum_h sum_m q[*,diag]k[*,diag] via a
     strided DVE multiply + free-axis reduce + selector matmul;
     rank-1 term outer = vsum (x) dv   (vsum = column sums of v via PE).
  4. gate: asym = |x_left - xflip_left| (xflip precomputed on host as a
     layout transform), g1 = gelu(Wg1@asym + b1), g = sigmoid(Wg2@. + b2)
     via exp, expanded pairwise to full width (adaptive pool w/2 -> w).
  5. out_merged = av * (1/r)(1+g) + outer*(1+g);  final conv W_out + b_out.
"""

import os
import sys

import numpy as np

for _p in ("/root/.axon_site", "/root/.axon_site/_ro/trn_rl_repo",
           "/root/.axon_site/_ro/pypackages", "/opt/trn_rl_repo", "/opt/pypackages"):
    if os.path.isdir(_p) and _p not in sys.path:
        sys.path.append(_p)

HEADS = 8
DIM = 256
D = 32          # head dim
NPIX = 1024     # 32*32
P = 128
SCALE = D ** -0.5
DIAG_W = 0.3

_CACHE = {}


def _build_program():
    import concourse.tile as tile
    from concourse import bacc, mybir

    f32 = mybir.dt.float32
    AF = mybir.ActivationFunctionType
    ALU = mybir.AluOpType

    nc = bacc.Bacc("TRN2", target_bir_lowering=False, debug=False)

    # ---- DRAM I/O ----
    x_d = nc.dram_tensor("x", [DIM, NPIX], f32, kind="ExternalInput").ap()
    xf_d = nc.dram_tensor("xflip", [DIM, NPIX], f32, kind="ExternalInput").ap()
    wqk_d = nc.dram_tensor("wqkT", [DIM, 512], f32, kind="ExternalInput").ap()
    wv_d = nc.dram_tensor("wvT", [DIM, DIM], f32, kind="ExternalInput").ap()
    wo_d = nc.dram_tensor("woutT", [DIM, DIM], f32, kind="ExternalInput").ap()
    bo_d = nc.dram_tensor("b_out", [DIM, 1], f32, kind="ExternalInput").ap()
    wg1_d = nc.dram_tensor("wg1T", [DIM, 64], f32, kind="ExternalInput").ap()
    bg1_d = nc.dram_tensor("b_g1", [64, 1], f32, kind="ExternalInput").ap()
    wg2_d = nc.dram_tensor("wg2T", [64, 1], f32, kind="ExternalInput").ap()
    nbg2_d = nc.dram_tensor("nb_g2", [1, 1], f32, kind="ExternalInput").ap()
    sel_d = nc.dram_tensor("sel", [P, D], f32, kind="ExternalInput").ap()
    out_d = nc.dram_tensor("out", [DIM, NPIX], f32, kind="ExternalOutput").ap()

    with tile.TileContext(nc) as tc:
        _body(nc, tc, tile, mybir, f32, AF, ALU,
              x_d, xf_d, wqk_d, wv_d, wo_d, bo_d, wg1_d, bg1_d, wg2_d, nbg2_d,
              sel_d, out_d)
    nc.compile()
    return nc


def _body(nc, tc, tile, mybir, f32, AF, ALU,
          x_d, xf_d, wqk_d, wv_d, wo_d, bo_d, wg1_d, bg1_d, wg2_d, nbg2_d,
          sel_d, out_d):
    import concourse.bass as bass
    from contextlib import ExitStack

    ctx = ExitStack()
    with ctx:
        consts = ctx.enter_context(tc.tile_pool(name="consts", bufs=1))
        work = ctx.enter_context(tc.tile_pool(name="work", bufs=1))
        epool = ctx.enter_context(tc.tile_pool(name="epool", bufs=3))
        rows = ctx.enter_context(tc.tile_pool(name="rows", bufs=8))
        drs = ctx.enter_context(tc.tile_pool(name="drs", bufs=1, space="DRAM"))
        g_dram = drs.tile([1, NPIX], f32, name="g_dram")
        dv_dram = drs.tile([1, D], f32, name="dv_dram")
        r_dram = drs.tile([8, NPIX], f32, name="r_dram")

        # ---- load constants / inputs ----
        x_sb = [consts.tile([P, NPIX], f32, name=f"x{k}") for k in range(2)]
        xf_sb = [consts.tile([P, NPIX], f32, name=f"xf{k}") for k in range(2)]
        wqk_sb = [consts.tile([P, 512], f32, name=f"wqk{k}") for k in range(2)]
        wv_sb = [consts.tile([P, DIM], f32, name=f"wv{k}") for k in range(2)]
        wo_sb = [consts.tile([P, DIM], f32, name=f"wo{k}") for k in range(2)]
        wg1_sb = [consts.tile([P, 64], f32, name=f"wg1{k}") for k in range(2)]
        bo_sb = [consts.tile([P, 1], f32, name=f"bo{k}") for k in range(2)]
        bg1_sb = consts.tile([64, 1], f32, name="bg1")
        wg2_sb = consts.tile([64, 1], f32, name="wg2")
        nbg2_sb = consts.tile([1, 1], f32, name="nbg2")
        sel_sb = consts.tile([P, D], f32, name="sel")
        ones_sb = consts.tile([P, 1], f32, name="ones")

        for k in range(2):
            # SWDGE: one software queue -> one semaphore, so the DVE
            # consumer of x/xf stays within walrus' sync-wait slot limit
            nc.gpsimd.dma_start(x_sb[k], x_d[k * P:(k + 1) * P, :])
            nc.gpsimd.dma_start(xf_sb[k], xf_d[k * P:(k + 1) * P, :])
            nc.sync.dma_start(wqk_sb[k], wqk_d[k * P:(k + 1) * P, :])
            nc.sync.dma_start(wv_sb[k], wv_d[k * P:(k + 1) * P, :])
            nc.sync.dma_start(wo_sb[k], wo_d[k * P:(k + 1) * P, :])
            nc.sync.dma_start(wg1_sb[k], wg1_d[k * P:(k + 1) * P, :])
            nc.sync.dma_start(bo_sb[k], bo_d[k * P:(k + 1) * P, :])
        nc.sync.dma_start(bg1_sb, bg1_d)
        nc.sync.dma_start(wg2_sb, wg2_d)
        nc.sync.dma_start(nbg2_sb, nbg2_d)
        nc.sync.dma_start(sel_sb, sel_d)
        nc.vector.memset(ones_sb, 1.0)

        # persistent SBUF results
        qk_sb = [consts.tile([P, NPIX], f32, name=f"qk{m}") for m in range(4)]
        vt_sb = [consts.tile([P, DIM], f32, name=f"vt{j}") for j in range(8)]
        vsum_sb = consts.tile([1, DIM], f32, name="vsum")
        dvrow_sb = consts.tile([1, D], f32, name="dvrow")
        outer_sb = [consts.tile([P, D], f32, name=f"outer{g}") for g in range(2)]
        gfull_sb = consts.tile([1, NPIX], f32, name="gfull")
        grep_sb = consts.tile([P, D], f32, name="grep")
        mult_sb = [consts.tile([P, NPIX], f32, name=f"mult{g}") for g in range(2)]
        om_sb = [consts.tile([P, NPIX], f32, name=f"om{g}") for g in range(2)]
        fin_sb = [consts.tile([P, NPIX], f32, name=f"fin{m}") for m in range(2)]

        # =========== Phase A: qkv, vtrans, diag, gate ===========
        with tc.tile_pool(name="psA", bufs=4, space="PSUM") as psA:
            # --- gate branch (do its ACT work first: gelu table, then exp) ---
            asym_sb = [work.tile([P, 512], f32, name=f"asym{k}") for k in range(2)]
            for k in range(2):
                dtile = work.tile([P, NPIX], f32, name=f"dtile{k}")
                # full-width 2D subtract (3D TT encodings have too few
                # sync-wait slots for walrus when fed straight from DMAs)
                nc.vector.tensor_sub(dtile, x_sb[k], xf_sb[k])
                dview = dtile.rearrange("p (y x) -> p y x", x=32)[:, :, 0:16]
                # |d| = max(-d, d), gathered to compact [128, 512]
                nc.vector.scalar_tensor_tensor(
                    out=asym_sb[k].rearrange("p (y x) -> p y x", x=16),
                    in0=dview, scalar=-1.0, in1=dview,
                    op0=ALU.mult, op1=ALU.max)

            g1_ps = psA.tile([64, 512], f32, tag="ps", name="g1_ps")
            for k in range(2):
                nc.tensor.matmul(g1_ps, lhsT=wg1_sb[k], rhs=asym_sb[k],
                                 start=(k == 0), stop=(k == 1))
            # tanh-form gelu (keeps the whole kernel on the exp/tanh ACT table;
            # |err| vs erf-gelu ~1e-3 pre-g2-conv -> ~1e-6 on the final output)
            u_sb = work.tile([64, 512], f32, name="u")
            nc.vector.tensor_scalar_add(u_sb, g1_ps, bg1_sb)
            t1_sb = work.tile([64, 512], f32, name="t1")
            nc.vector.tensor_mul(t1_sb, u_sb, u_sb)
            nc.vector.tensor_mul(t1_sb, t1_sb, u_sb)          # u^3
            # z = u + 0.044715*u^3
            nc.vector.scalar_tensor_tensor(
                out=t1_sb, in0=t1_sb, scalar=0.044715, in1=u_sb,
                op0=ALU.mult, op1=ALU.add)
            th_sb = work.tile([64, 512], f32, name="th")
            nc.scalar.activation(th_sb, t1_sb, AF.Tanh, scale=0.7978845608028654)
            y1_sb = work.tile([64, 512], f32, name="y1")
            # y1 = u*(1+tanh) = 2*gelu (the 0.5 is folded into wg2T on host)
            nc.vector.tensor_mul(th_sb, th_sb, u_sb)
            nc.vector.tensor_add(y1_sb, u_sb, th_sb)

            g2_ps = psA.tile([1, 512], f32, tag="ps", name="g2_ps")
            nc.tensor.matmul(g2_ps, lhsT=wg2_sb, rhs=y1_sb, start=True, stop=True)
            # sigmoid via exp table: s = 1/(1+exp(-(z+b))); gp = 1+s
            eg_sb = rows.tile([1, 512], f32, name="eg")
            nc.scalar.activation(eg_sb, g2_ps, AF.Exp, bias=nbg2_sb, scale=-1.0)
            nc.vector.tensor_single_scalar(eg_sb, eg_sb, 1.0, ALU.add)
            rec_sb = rows.tile([1, 512], f32, name="rec")
            nc.vector.reciprocal(rec_sb, eg_sb)
            gp_sb = rows.tile([1, 512], f32, name="gp")
            nc.vector.tensor_single_scalar(gp_sb, rec_sb, 1.0, ALU.add)
            # expand pairwise to [1, 1024]
            nc.vector.tensor_copy(gfull_sb[:, 0:NPIX:2], gp_sb)
            nc.vector.tensor_copy(gfull_sb[:, 1:NPIX:2], gp_sb)
            # grep = broadcast of gfull[0:32] over 128 partitions (via DRAM)
            nc.sync.dma_start(out=g_dram, in_=gfull_sb)
            nc.sync.dma_start(out=grep_sb,
                              in_=g_dram[:, 0:D].to_broadcast((P, D)))

            # --- qT / kT ---
            for m in range(4):
                for n in range(2):
                    ps = psA.tile([P, 512], f32, tag="ps", name=f"qk_ps{m}{n}")
                    for k in range(2):
                        nc.tensor.matmul(
                            ps,
                            lhsT=wqk_sb[k][:, m * P:(m + 1) * P],
                            rhs=x_sb[k][:, n * 512:(n + 1) * 512],
                            start=(k == 0), stop=(k == 1))
                    nc.vector.tensor_copy(qk_sb[m][:, n * 512:(n + 1) * 512], ps)

            # --- vtrans [1024, 256] in 8 chunks of [128, 256] ---
            for j in range(8):
                ps = psA.tile([P, DIM], f32, tag="ps", name=f"vt_ps{j}")
                for k in range(2):
                    nc.tensor.matmul(
                        ps,
                        lhsT=x_sb[k][:, j * P:(j + 1) * P],
                        rhs=wv_sb[k],
                        start=(k == 0), stop=(k == 1))
                nc.vector.tensor_copy(vt_sb[j], ps)

            # --- vsum[1, 256] = column sums of vtrans ---
            vs_ps = psA.tile([1, DIM], f32, tag="ps", name="vs_ps")
            for j in range(8):
                nc.tensor.matmul(vs_ps, lhsT=ones_sb, rhs=vt_sb[j],
                                 start=(j == 0), stop=(j == 7))
            nc.vector.tensor_copy(vsum_sb, vs_ps)

            # --- diag vector dv ---
            schan_sb = [rows.tile([P, 1], f32, name=f"schan{g}") for g in range(2)]
            for g in range(2):
                prod = work.tile([P, D], f32, name=f"prod{g}")
                nc.vector.tensor_mul(prod,
                                     qk_sb[g][:, 0:NPIX:33],
                                     qk_sb[2 + g][:, 0:NPIX:33])
                nc.vector.reduce_sum(schan_sb[g], prod, axis=mybir.AxisListType.X)
            dv_ps = psA.tile([D, 1], f32, tag="ps", name="dv_ps")
            for g in range(2):
                nc.tensor.matmul(dv_ps, lhsT=sel_sb, rhs=schan_sb[g],
                                 start=(g == 0), stop=(g == 1))
            dv_sb = rows.tile([D, 1], f32, name="dv")
            nc.vector.tensor_copy(dv_sb, dv_ps)
            # transpose [32,1] -> [1,32] via DRAM round-trip
            nc.sync.dma_start(out=dv_dram, in_=dv_sb)
            nc.sync.dma_start(out=dvrow_sb, in_=dv_dram)

            # --- outer_g = (vsum_g (x) dv) * grep ---
            for g in range(2):
                o_ps = psA.tile([P, D], f32, tag="ps", name=f"o_ps{g}")
                nc.tensor.matmul(o_ps, lhsT=vsum_sb[:, g * P:(g + 1) * P],
                                 rhs=dvrow_sb, start=True, stop=True)
                nc.vector.tensor_mul(outer_sb[g], o_ps, grep_sb)

        # =========== Phase B: attention ===========
        with tc.tile_pool(name="psS", bufs=1, space="PSUM") as psS, \
             tc.tile_pool(name="psAV", bufs=1, space="PSUM") as psAV:
            for g in range(2):
                av_ps = psAV.tile([P, 2048], f32, tag="av", name=f"av_ps{g}")
                for jt in range(8):
                    for ih in range(2):
                        sc = psS.tile([P, 2048], f32, tag="sc",
                                      name=f"sc{g}{jt}{ih}")
                        for hh in range(4):
                            nc.tensor.matmul(
                                sc[:, hh * 512:(hh + 1) * 512],
                                lhsT=qk_sb[2 + g][hh * D:(hh + 1) * D,
                                                  jt * P:(jt + 1) * P],
                                rhs=qk_sb[g][hh * D:(hh + 1) * D,
                                             ih * 512:(ih + 1) * 512],
                                start=True, stop=True,
                                tile_position=(hh * D, 0))
                        e = epool.tile([P, 2048], f32, tag="e",
                                       name=f"e{g}{jt}{ih}")
                        nc.scalar.activation(e, sc, AF.Exp)
                        for hh in range(4):
                            nc.tensor.matmul(
                                av_ps[hh * D:(hh + 1) * D,
                                      ih * 512:(ih + 1) * 512],
                                lhsT=vt_sb[jt][:, g * P + hh * D:
                                               g * P + (hh + 1) * D],
                                rhs=e[:, hh * 512:(hh + 1) * 512],
                                start=(jt == 0), stop=(jt == 7),
                                tile_position=(0, hh * D),
                                skip_group_check=True)
                            nc.tensor.matmul(
                                av_ps[hh * D:hh * D + 1,
                                      1024 + ih * 512:1024 + (ih + 1) * 512],
                                lhsT=ones_sb,
                                rhs=e[:, hh * 512:(hh + 1) * 512],
                                start=(jt == 0), stop=(jt == 7),
                                tile_position=(0, hh * D),
                                skip_group_check=True)

                # normalize + gate for this group
                for hh in range(4):
                    rrow = rows.tile([1, NPIX], f32, tag="rr", name=f"rr{g}{hh}")
                    nc.vector.tensor_copy(rrow, av_ps[hh * D:hh * D + 1,
                                                      1024:2048])
                    nc.vector.reciprocal(rrow, rrow)
                    nc.vector.tensor_mul(rrow, rrow, gfull_sb)
                    idx = g * 4 + hh
                    nc.sync.dma_start(out=r_dram[idx:idx + 1, :], in_=rrow)
                # one broadcast DMA per group: row hh -> partitions 32hh..32hh+32
                src = r_dram[g * 4:(g + 1) * 4, :]
                nc.gpsimd.dma_start(
                    out=mult_sb[g],
                    in_=bass.AP(tensor=src.tensor, offset=src.offset,
                                ap=[[NPIX, 4], [0, D], [1, NPIX]]))
                nc.vector.tensor_mul(om_sb[g], av_ps[:, 0:1024], mult_sb[g])
                nc.vector.tensor_add(om_sb[g][:, 0:D], om_sb[g][:, 0:D],
                                     outer_sb[g])

        # =========== Phase C: final 1x1 conv ===========
        with tc.tile_pool(name="psC", bufs=4, space="PSUM") as psC:
            for m in range(2):
                for n in range(2):
                    ps = psC.tile([P, 512], f32, tag="ps", name=f"fin_ps{m}{n}")
                    for k in range(2):
                        nc.tensor.matmul(
                            ps,
                            lhsT=wo_sb[k][:, m * P:(m + 1) * P],
                            rhs=om_sb[k][:, n * 512:(n + 1) * 512],
                            start=(k == 0), stop=(k == 1))
                    nc.vector.tensor_scalar_add(
                        fin_sb[m][:, n * 512:(n + 1) * 512], ps, bo_sb[m])
                nc.sync.dma_start(out_d[m * P:(m + 1) * P, :], fin_sb[m])


def _prep_inputs(x, w_qkv, w_out, b_out, w_g1, b_g1, w_g2, b_g2):
    """Per-core input maps (host-side layout prep only)."""
    x = np.asarray(x, np.float32)
    w_qkv = np.asarray(w_qkv, np.float32)
    wqkT = np.ascontiguousarray(w_qkv[:512].T)        # [256, 512]
    wqkT = wqkT.copy()
    wqkT[:, 0:256] *= SCALE                           # fold attn scale into Wq
    wvT = np.ascontiguousarray(w_qkv[512:768].T)      # [256, 256]
    woT = np.ascontiguousarray(np.asarray(w_out, np.float32).T)
    bo = np.asarray(b_out, np.float32).reshape(DIM, 1)
    wg1T = np.ascontiguousarray(np.asarray(w_g1, np.float32).T)  # [256, 64]
    bg1 = np.asarray(b_g1, np.float32).reshape(64, 1)
    wg2T = np.ascontiguousarray((0.5 * np.asarray(w_g2, np.float32)).T)  # [64,1]
    nbg2 = (-np.asarray(b_g2, np.float32)).reshape(1, 1)
    sel = np.zeros((P, D), np.float32)
    for ch in range(P):
        sel[ch, ch % D] = DIAG_W / HEADS
    shared = dict(wqkT=wqkT, wvT=wvT, woutT=woT, b_out=bo, wg1T=wg1T,
                  b_g1=bg1, wg2T=wg2T, nb_g2=nbg2, sel=sel)

    in_maps = []
    for i in range(x.shape[0]):
        xi = np.ascontiguousarray(x[i].reshape(DIM, NPIX))
        xfi = np.ascontiguousarray(x[i, :, :, ::-1].reshape(DIM, NPIX))
        m = dict(shared)
        m["x"] = xi
        m["xflip"] = xfi
        in_maps.append(m)
    return in_maps


def _get_nc():
    if "nc" not in _CACHE:
        _CACHE["nc"] = _build_program()
    return _CACHE["nc"]


def kernel(x, w_qkv, w_out, b_out, w_g1, b_g1, w_g2, b_g2, **kw):
    from concourse import bass_utils

    nc = _get_nc()
    in_maps = _prep_inputs(x, w_qkv, w_out, b_out, w_g1, b_g1, w_g2, b_g2)
    n = len(in_maps)
    res = bass_utils.run_bass_kernel_spmd(nc, in_maps, core_ids=list(range(n)))
    outs = [res.results[i]["out"].reshape(DIM, 32, 32) for i in range(n)]
    return np.stack(outs, axis=0).astype(np.float32)
